# revision 27
# baseline (speedup 1.0000x reference)
"""Trainium2 Bass kernel for GQA attention (B=2, S=2048, D=2048, H=16, KVH=8, HD=128).

Sharding: tensor-parallel over heads (4 groups of 4 q-heads / 2 kv-heads) x
data-parallel over batch (2) = 8 cores. Each core computes a partial output
(full rows for its batch, its head-group's contribution through wo); the host
sums the 4 partials per batch.

v3 dataflow (all matmul operands bf16, PSUM f32):
  - Software-pipelined: attention for q-block qc interleaves with the QKV
    projection of q-block qc+1 (one sequence chunk per head), then the output
    projection for qc's panel.
  - Each roped half transposes to head-dim-major layout with ONE DMA-xbar
    transpose (3D out AP maps logical row h*128+hd, matching the natural
    (h, hd) column order), dispatched on the scalar queue so the sync queue
    stays free for loads/stores.
  - RoPE on DVE straight out of PSUM (bf16 out); V natural via DVE copy.
  - Attention transposed: S^T tiles = K-blk stationary @ Q^T moving, exp on
    ACT (scale folded), causal handled by computing only the valid q-suffix
    of diagonal tiles + one [128,128] triangle mask multiply on DVE.
  - Softmax denominators accumulated on DVE (bf16); one ones-vector matmul
    folds partitions, ACT copy + PE broadcast + DVE reciprocal + multiply.
  - PV accumulated in PSUM with V stationary, ascending k so the first
    (full-width) tile opens the accumulation group.
  - bf16 output partials, summed in f32 on the host.
"""

import math

import numpy as np
import ml_dtypes

import concourse.bass as bass
import concourse.mybir as mybir
import concourse.tile as tile
from concourse import bacc
from concourse.bass_utils import run_bass_kernel_spmd

F32 = mybir.dt.float32
F32R = mybir.dt.float32r
BF16 = mybir.dt.bfloat16

B, S, D = 2, 2048, 2048
H, KVH, HD = 16, 8, 128
TP, DP = 4, 2
HL = H // TP        # 4 q heads per core
KVL = KVH // TP     # 2 kv heads per core
NQ = HL * HD        # 512 q cols per core
NKV = KVL * HD      # 256 k (and v) cols per core
NW = NQ + 2 * NKV   # 1024 fused qkv cols per core
NSC = S // 128      # 16 sequence chunks of 128
NKC = D // 128      # 16 contraction chunks of 128
NQC = S // 512      # 4 q chunks of 512
SCALE = 1.0 / math.sqrt(HD)

_BUILT = None


def _build():
    nc = bacc.Bacc("TRN2", target_bir_lowering=False, debug=False)

    xt_d = nc.dram_tensor("xt", (NSC, 128, NKC, 128), BF16, kind="ExternalInput")
    w_d = nc.dram_tensor("w", (128, NKC, NW), BF16, kind="ExternalInput")
    wo_d = nc.dram_tensor("wo", (128, HL, D), BF16, kind="ExternalInput")
    sn_d = nc.dram_tensor("sn", (NSC, 128, HD), BF16, kind="ExternalInput")
    cpm_d = nc.dram_tensor("cpm", (NSC, 128, HD), BF16, kind="ExternalInput")
    m128_d = nc.dram_tensor("m128", (128, 128), BF16, kind="ExternalInput")
    onec_d = nc.dram_tensor("onec", (128, 1), BF16, kind="ExternalInput")
    oner_d = nc.dram_tensor("oner", (1, 128), F32R, kind="ExternalInput")
    out_d = nc.dram_tensor("out", (S, D), BF16, kind="ExternalOutput")

    EXP = mybir.ActivationFunctionType.Exp

    with tile.TileContext(nc) as tc:
        with (
            nc.allow_low_precision(reason="bf16 dataflow is intentional"),
            tc.tile_pool(name="consts", bufs=1) as consts,
            tc.tile_pool(name="res", bufs=1) as res,
            tc.tile_pool(name="ares", bufs=2) as ares,
            tc.tile_pool(name="xtp", bufs=3) as xtp,
            tc.tile_pool(name="tabs", bufs=3) as tabs,
            tc.tile_pool(name="rw", bufs=2) as rw,
            tc.tile_pool(name="pep", bufs=6) as pep,
            tc.tile_pool(name="accp", bufs=2) as accp,
            tc.tile_pool(name="rip", bufs=2) as rip,
            tc.tile_pool(name="osp", bufs=3) as osp,
            tc.tile_pool(name="psA", bufs=2, space="PSUM") as psA,
            tc.tile_pool(name="psB", bufs=3, space="PSUM") as psB,
            tc.tile_pool(name="psC", bufs=2, space="PSUM") as psC,
            tc.tile_pool(name="psD", bufs=1, space="PSUM") as psD,
        ):
            m128 = consts.tile([128, 128], BF16, name="m128")
            nc.sync.dma_start(m128[:], m128_d.ap())
            onec = consts.tile([128, 1], BF16, name="onec")
            nc.sync.dma_start(onec[:], onec_d.ap())
            oner = consts.tile([1, 128], F32R, name="oner")
            nc.sync.dma_start(oner[:], oner_d.ap())

            wt = {
                c: consts.tile([128, NW], BF16, name=f"w_{c}")
                for c in range(NKC)
            }
            wo = consts.tile([128, HL, D], BF16, name="wo_t")

            # Residents: head-dim-major Q^T/K^T per q-block, natural V rows.
            qT = {}
            kT = {}
            for qc in range(NQC):
                qT[qc] = res.tile(
                    [128, HL, 512], BF16, tag=f"qT_{qc}", name=f"qT_{qc}"
                )
                kT[qc] = res.tile(
                    [128, KVL, 512], BF16, tag=f"kT_{qc}", name=f"kT_{qc}"
                )
            vn = {}
            for sc in range(NSC):
                vn[sc] = res.tile([128, NKV], BF16, tag=f"v_{sc}", name=f"v_{sc}")

            def load_sc(sc):
                xt = xtp.tile([128, NKC, 128], BF16, tag="xt", name="xt")
                eng = nc.gpsimd if sc > 0 else nc.sync
                eng.dma_start(xt[:, 0:8, :], xt_d.ap()[sc][:, 0:8, :])
                eng.dma_start(xt[:, 8:16, :], xt_d.ap()[sc][:, 8:16, :])
                sn = tabs.tile([128, HD], BF16, tag="sn", name="sn")
                nc.sync.dma_start(sn[:], sn_d.ap()[sc])
                cpm = tabs.tile([128, HD], BF16, tag="cpm", name="cpm")
                nc.sync.dma_start(cpm[:], cpm_d.ap()[sc])
                return xt, sn, cpm

            def rope_half(ph, nheads, sn, cpm):
                """RoPE the first nheads*HD cols (head-minor packed) -> bf16."""
                nrope = nheads * HD
                ph4 = ph[:, 0:nrope].rearrange(
                    "p (h i two) -> p h i two", h=nheads, two=2
                )
                cpm3 = cpm[:].rearrange("p (i two) -> p i two", two=2)
                cpm_e = cpm3[:, :, 0].unsqueeze(1).broadcast_to(
                    [128, nheads, HD // 2]
                )
                cpm_o = cpm3[:, :, 1].unsqueeze(1).broadcast_to(
                    [128, nheads, HD // 2]
                )
                sn_b = sn[:].unsqueeze(1).broadcast_to([128, nheads, HD])
                swp = rw.tile([128, nrope], BF16, tag=f"swp{nheads}", name="swp")
                swp4 = swp[:].rearrange(
                    "p (h i two) -> p h i two", h=nheads, two=2
                )
                nc.vector.tensor_mul(swp4[:, :, :, 0], ph4[:, :, :, 1], cpm_e)
                nc.vector.tensor_mul(swp4[:, :, :, 1], ph4[:, :, :, 0], cpm_o)
                t1 = rw.tile([128, nrope], BF16, tag=f"t1{nheads}", name="t1")
                nc.vector.tensor_mul(
                    t1[:].rearrange("p (h d) -> p h d", h=nheads),
                    ph[:, 0:nrope].rearrange("p (h d) -> p h d", h=nheads),
                    sn_b,
                )
                roped = rw.tile(
                    [128, nrope], BF16, tag=f"rp{nheads}", name="roped"
                )
                nc.vector.tensor_add(roped[:], t1[:], swp[:])
                return roped

            state = {"nxt": load_sc(0)}
            for c in range(NKC):
                eng = nc.sync if c % 2 == 0 else nc.scalar
                eng.dma_start(wt[c][:], w_d.ap()[:, c, :])
            for hh in range(HL):
                nc.scalar.dma_start(wo[:, hh, :], wo_d.ap()[:, hh, :])

            def project_sc_gen(sc):
                """QKV projection + RoPE + transposes for sequence chunk sc.

                Yields after each contraction-chunk matmul pair so the caller
                can interleave these PE-only ops between attention tiles
                (which are paced by ACT's exp) to keep the PE fed.
                """
                qc = sc // 4
                xt, sn, cpm = state["nxt"]
                if sc + 1 < NSC:
                    state["nxt"] = load_sc(sc + 1)
                ph0 = psA.tile([128, NQ], F32, tag="ph", name="ph0")
                ph1 = psA.tile([128, 2 * NKV], F32, tag="ph", name="ph1")
                for c in range(NKC):
                    nc.tensor.matmul(
                        ph0[:], xt[:, c, :], wt[c][:, 0:NQ],
                        start=(c == 0), stop=(c == NKC - 1),
                    )
                    nc.tensor.matmul(
                        ph1[:], xt[:, c, :], wt[c][:, NQ:NW],
                        start=(c == 0), stop=(c == NKC - 1),
                    )
                    yield
                ropq = rope_half(ph0, HL, sn, cpm)
                nc.sync.dma_start_transpose(
                    qT[qc][:, :, (sc % 4) * 128:(sc % 4 + 1) * 128], ropq[:]
                )
                ropk = rope_half(ph1, KVL, sn, cpm)
                nc.sync.dma_start_transpose(
                    kT[qc][:, :, (sc % 4) * 128:(sc % 4 + 1) * 128], ropk[:]
                )
                nc.vector.tensor_copy(vn[sc][:], ph1[:, NKV:2 * NKV])

            def project_sc(sc):
                for _ in project_sc_gen(sc):
                    pass

            def attention(h, qc, filler=None):
                j = h // 2
                kend = 4 * qc + 4
                pv = psC.tile([128, 512], F32, tag="pv", name="pv")
                acc = accp.tile([128, 512], BF16, tag="acc", name="acc")
                if h < HL - 1:
                    nfeed = {0: 5, 1: 3}.get(qc, 2)
                else:
                    nfeed = 1
                for ki, kc in enumerate(range(kend)):
                    if filler is not None:
                        filler.feed(nfeed)
                    q0 = 128 * (kc - 4 * qc) if kc >= 4 * qc else 0
                    sp = psB.tile([128, 512], F32, tag="sp", name="sp")
                    nc.tensor.matmul(
                        sp[:, q0:512],
                        kT[kc // 4][:, j, (kc % 4) * 128:(kc % 4 + 1) * 128],
                        qT[qc][:, h, q0:512],
                        start=True, stop=True,
                    )
                    pe = pep.tile([128, 512], BF16, tag="pe", name="pe")
                    nc.scalar.activation(
                        pe[:, q0:512], sp[:, q0:512], EXP, scale=SCALE
                    )
                    if kc >= 4 * qc:
                        nc.vector.tensor_mul(
                            pe[:, q0:q0 + 128], pe[:, q0:q0 + 128], m128[:]
                        )
                    nc.tensor.matmul(
                        pv[:, q0:512],
                        vn[kc][:, j * 128:(j + 1) * 128],
                        pe[:, q0:512],
                        start=(ki == 0), stop=(ki == kend - 1),
                        skip_group_check=True,
                    )
                    if ki == 0:
                        nc.vector.tensor_copy(acc[:], pe[:])
                    else:
                        nc.vector.tensor_add(
                            acc[:, q0:512], acc[:, q0:512], pe[:, q0:512]
                        )
                if filler is not None:
                    filler.feed(2)
                rbb = psD.tile([128, 512], F32, tag="rbb", name="rbb")
                nc.tensor.matmul(
                    rbb[0:1, :], onec[:], acc[:], start=True, stop=True
                )
                rs_sb = rip.tile([1, 512], F32R, tag="ri", name="rs_sb")
                nc.scalar.copy(rs_sb[:], rbb[0:1, :])
                if filler is not None:
                    filler.feed(1)
                nc.tensor.matmul(rbb[:], oner[:], rs_sb[:], start=True, stop=True)
                rb = rip.tile([128, 512], F32, tag="rb", name="rb")
                nc.vector.reciprocal_approx_fast(out=rb[:], in_=rbb[:])
                aT = ares.tile([128, 512], BF16, tag=f"aT{h}", name=f"aT{h}")
                nc.vector.tensor_mul(aT[:], pv[:], rb[:])
                return aT

            def p3_gen(qc, aTc):
                """Output projection for q-panel qc; yields per (qp, dc) unit."""
                idx = 0
                for qp in range(4 * qc, 4 * qc + 4):
                    qoff = (qp % 4) * 128
                    for dc in range(4):
                        idx += 1
                        po = psB.tile([128, 512], F32, tag="sp", name="po")
                        for h in range(HL):
                            nc.tensor.matmul(
                                po[:],
                                aTc[h][:, qoff:qoff + 128],
                                wo[:, h, dc * 512:(dc + 1) * 512],
                                start=(h == 0), stop=(h == HL - 1),
                            )
                        osb = osp.tile([128, 512], BF16, tag="osb", name="osb")
                        on_act = (
                            (qp + dc) % 2 == 0 if qc == NQC - 1 else idx > 8
                        )
                        if on_act:
                            nc.scalar.copy(osb[:], po[:])
                        else:
                            nc.vector.tensor_copy(osb[:], po[:])
                        oeng = (
                            nc.sync if qc == NQC - 1 and (qp + dc) % 2 == 0
                            else nc.gpsimd
                        )
                        oeng.dma_start(
                            out_d.ap()[qp * 128:(qp + 1) * 128,
                                       dc * 512:(dc + 1) * 512],
                            osb[:],
                        )
                        yield

            class FillerQueue:
                def __init__(self):
                    self.gens = []

                def add(self, gen):
                    self.gens.append(gen)

                def feed(self, k=1):
                    for _ in range(k):
                        while self.gens:
                            try:
                                next(self.gens[0])
                                break
                            except StopIteration:
                                self.gens.pop(0)
                        else:
                            return

                def drain(self):
                    while self.gens:
                        for _ in self.gens.pop(0):
                            pass

            fq = FillerQueue()
            for sc in range(4):
                project_sc(sc)
            for qc in range(NQC):
                aTc = {}
                for h in range(HL):
                    if qc + 1 < NQC:
                        fq.add(project_sc_gen(4 * (qc + 1) + h))
                    aTc[h] = attention(h, qc, fq)
                fq.drain()
                fq.add(p3_gen(qc, aTc))
            fq.drain()

    nc.compile()
    return nc


def _get_nc():
    global _BUILT
    if _BUILT is None:
        _BUILT = _build()
    return _BUILT


def _host_prep(x, freqs_cis, wq, wk, wv, wo):
    """Build the 8 per-core input maps (bf16)."""
    bf = ml_dtypes.bfloat16
    f = np.asarray(freqs_cis, dtype=np.float32)
    sn = np.repeat(f[:, :, 1], 2, axis=1)                    # (S, HD)
    cos = f[:, :, 0]
    cpm = np.empty((S, HD), dtype=np.float32)
    cpm[:, 0::2] = -cos
    cpm[:, 1::2] = cos
    sn_t = np.ascontiguousarray(sn.reshape(NSC, 128, HD)).astype(bf)
    cpm_t = np.ascontiguousarray(cpm.reshape(NSC, 128, HD)).astype(bf)

    kp = np.arange(128)[:, None]
    qf = np.arange(128)[None, :]
    m128 = (kp <= qf).astype(np.float32).astype(bf)           # (128,128)

    onec = np.ones((128, 1), dtype=np.float32).astype(bf)
    oner = np.ones((1, 128), dtype=np.float32)

    xts = []
    for b in range(DP):
        xb = np.asarray(x[b], dtype=np.float32)              # (S, D)
        x4 = xb.reshape(NSC, 128, NKC, 128).transpose(0, 3, 2, 1)
        xts.append(np.ascontiguousarray(x4).astype(bf))      # (NSC,128,NKC,128)

    wq = np.asarray(wq, dtype=np.float32)
    wk = np.asarray(wk, dtype=np.float32)
    wv = np.asarray(wv, dtype=np.float32)
    wo = np.asarray(wo, dtype=np.float32)

    in_maps = []
    for c in range(8):
        b, g = c // TP, c % TP
        w_all = np.concatenate(
            [
                wq[:, g * NQ:(g + 1) * NQ],
                wk[:, g * NKV:(g + 1) * NKV],
                wv[:, g * NKV:(g + 1) * NKV],
            ],
            axis=1,
        )  # (D, NW)
        w_t = np.ascontiguousarray(
            w_all.reshape(NKC, 128, NW).transpose(1, 0, 2)
        ).astype(bf)
        wo_g = wo[g * NQ:(g + 1) * NQ, :]                    # (NQ, D)
        wo_t = np.ascontiguousarray(
            wo_g.reshape(HL, 128, D).transpose(1, 0, 2)
        ).astype(bf)
        in_maps.append(
            {
                "xt": xts[b],
                "w": w_t,
                "wo": wo_t,
                "sn": sn_t,
                "cpm": cpm_t,
                "m128": m128,
                "onec": onec,
                "oner": oner,
            }
        )
    return in_maps


def kernel(x, freqs_cis, mask, wq, wk, wv, wo, _trace=False, _tmpdir=None):
    nc = _get_nc()
    in_maps = _host_prep(x, freqs_cis, wq, wk, wv, wo)
    res = run_bass_kernel_spmd(
        nc, in_maps, core_ids=list(range(8)), trace=_trace, tmpdir=_tmpdir
    )
    out = np.empty((B, S, D), dtype=np.float32)
    for b in range(DP):
        acc = res.results[b * TP + 0]["out"].astype(np.float32)
        for g in range(1, TP):
            acc = acc + res.results[b * TP + g]["out"].astype(np.float32)
        out[b] = acc
    kernel._last_results = res
    return out


# revision 28
# speedup vs baseline: 1.0054x; 1.0054x over previous
"""Trainium2 Bass kernel for GQA attention (B=2, S=2048, D=2048, H=16, KVH=8, HD=128).

Sharding: tensor-parallel over heads (4 groups of 4 q-heads / 2 kv-heads) x
data-parallel over batch (2) = 8 cores. Each core computes a partial output
(full rows for its batch, its head-group's contribution through wo); the host
sums the 4 partials per batch.

v3 dataflow (all matmul operands bf16, PSUM f32):
  - Software-pipelined: attention for q-block qc interleaves with the QKV
    projection of q-block qc+1 (one sequence chunk per head), then the output
    projection for qc's panel.
  - Each roped half transposes to head-dim-major layout with ONE DMA-xbar
    transpose (3D out AP maps logical row h*128+hd, matching the natural
    (h, hd) column order), dispatched on the scalar queue so the sync queue
    stays free for loads/stores.
  - RoPE on DVE straight out of PSUM (bf16 out); V natural via DVE copy.
  - Attention transposed: S^T tiles = K-blk stationary @ Q^T moving, exp on
    ACT (scale folded), causal handled by computing only the valid q-suffix
    of diagonal tiles + one [128,128] triangle mask multiply on DVE.
  - Softmax denominators accumulated on DVE (bf16); one ones-vector matmul
    folds partitions, ACT copy + PE broadcast + DVE reciprocal + multiply.
  - PV accumulated in PSUM with V stationary, ascending k so the first
    (full-width) tile opens the accumulation group.
  - bf16 output partials, summed in f32 on the host.
"""

import math

import numpy as np
import ml_dtypes

import concourse.bass as bass
import concourse.mybir as mybir
import concourse.tile as tile
from concourse import bacc
from concourse.bass_utils import run_bass_kernel_spmd

F32 = mybir.dt.float32
F32R = mybir.dt.float32r
BF16 = mybir.dt.bfloat16

B, S, D = 2, 2048, 2048
H, KVH, HD = 16, 8, 128
TP, DP = 4, 2
HL = H // TP        # 4 q heads per core
KVL = KVH // TP     # 2 kv heads per core
NQ = HL * HD        # 512 q cols per core
NKV = KVL * HD      # 256 k (and v) cols per core
NW = NQ + 2 * NKV   # 1024 fused qkv cols per core
NSC = S // 128      # 16 sequence chunks of 128
NKC = D // 128      # 16 contraction chunks of 128
NQC = S // 512      # 4 q chunks of 512
SCALE = 1.0 / math.sqrt(HD)

_BUILT = None


def _build():
    nc = bacc.Bacc("TRN2", target_bir_lowering=False, debug=False)

    xt_d = nc.dram_tensor("xt", (NSC, 128, NKC, 128), BF16, kind="ExternalInput")
    w_d = nc.dram_tensor("w", (128, NKC, NW), BF16, kind="ExternalInput")
    wo_d = nc.dram_tensor("wo", (128, HL, D), BF16, kind="ExternalInput")
    sn_d = nc.dram_tensor("sn", (NSC, 128, HD), BF16, kind="ExternalInput")
    cpm_d = nc.dram_tensor("cpm", (NSC, 128, HD), BF16, kind="ExternalInput")
    m128_d = nc.dram_tensor("m128", (128, 128), BF16, kind="ExternalInput")
    onec_d = nc.dram_tensor("onec", (128, 1), BF16, kind="ExternalInput")
    oner_d = nc.dram_tensor("oner", (1, 128), F32R, kind="ExternalInput")
    out_d = nc.dram_tensor("out", (S, D), BF16, kind="ExternalOutput")

    EXP = mybir.ActivationFunctionType.Exp

    with tile.TileContext(nc) as tc:
        with (
            nc.allow_low_precision(reason="bf16 dataflow is intentional"),
            tc.tile_pool(name="consts", bufs=1) as consts,
            tc.tile_pool(name="res", bufs=1) as res,
            tc.tile_pool(name="ares", bufs=2) as ares,
            tc.tile_pool(name="xtp", bufs=3) as xtp,
            tc.tile_pool(name="tabs", bufs=3) as tabs,
            tc.tile_pool(name="rw", bufs=2) as rw,
            tc.tile_pool(name="pep", bufs=6) as pep,
            tc.tile_pool(name="accp", bufs=2) as accp,
            tc.tile_pool(name="rip", bufs=2) as rip,
            tc.tile_pool(name="osp", bufs=3) as osp,
            tc.tile_pool(name="psA", bufs=2, space="PSUM") as psA,
            tc.tile_pool(name="psB", bufs=3, space="PSUM") as psB,
            tc.tile_pool(name="psC", bufs=2, space="PSUM") as psC,
            tc.tile_pool(name="psD", bufs=1, space="PSUM") as psD,
        ):

            wt = {
                c: consts.tile([128, NW], BF16, name=f"w_{c}")
                for c in range(NKC)
            }
            wo = consts.tile([128, HL, D], BF16, name="wo_t")

            # Residents: head-dim-major Q^T/K^T per q-block, natural V rows.
            qT = {}
            kT = {}
            for qc in range(NQC):
                qT[qc] = res.tile(
                    [128, HL, 512], BF16, tag=f"qT_{qc}", name=f"qT_{qc}"
                )
                kT[qc] = res.tile(
                    [128, KVL, 512], BF16, tag=f"kT_{qc}", name=f"kT_{qc}"
                )
            vn = {}
            for sc in range(NSC):
                vn[sc] = res.tile([128, NKV], BF16, tag=f"v_{sc}", name=f"v_{sc}")

            def load_sc(sc):
                xt = xtp.tile([128, NKC, 128], BF16, tag="xt", name="xt")
                eng = nc.gpsimd if sc > 0 else nc.sync
                eng.dma_start(xt[:, 0:8, :], xt_d.ap()[sc][:, 0:8, :])
                eng.dma_start(xt[:, 8:16, :], xt_d.ap()[sc][:, 8:16, :])
                sn = tabs.tile([128, HD], BF16, tag="sn", name="sn")
                nc.sync.dma_start(sn[:], sn_d.ap()[sc])
                cpm = tabs.tile([128, HD], BF16, tag="cpm", name="cpm")
                nc.sync.dma_start(cpm[:], cpm_d.ap()[sc])
                return xt, sn, cpm

            def rope_half(ph, nheads, sn, cpm):
                """RoPE the first nheads*HD cols (head-minor packed) -> bf16."""
                nrope = nheads * HD
                ph4 = ph[:, 0:nrope].rearrange(
                    "p (h i two) -> p h i two", h=nheads, two=2
                )
                cpm3 = cpm[:].rearrange("p (i two) -> p i two", two=2)
                cpm_e = cpm3[:, :, 0].unsqueeze(1).broadcast_to(
                    [128, nheads, HD // 2]
                )
                cpm_o = cpm3[:, :, 1].unsqueeze(1).broadcast_to(
                    [128, nheads, HD // 2]
                )
                sn_b = sn[:].unsqueeze(1).broadcast_to([128, nheads, HD])
                swp = rw.tile([128, nrope], BF16, tag=f"swp{nheads}", name="swp")
                swp4 = swp[:].rearrange(
                    "p (h i two) -> p h i two", h=nheads, two=2
                )
                nc.vector.tensor_mul(swp4[:, :, :, 0], ph4[:, :, :, 1], cpm_e)
                nc.vector.tensor_mul(swp4[:, :, :, 1], ph4[:, :, :, 0], cpm_o)
                t1 = rw.tile([128, nrope], BF16, tag=f"t1{nheads}", name="t1")
                nc.vector.tensor_mul(
                    t1[:].rearrange("p (h d) -> p h d", h=nheads),
                    ph[:, 0:nrope].rearrange("p (h d) -> p h d", h=nheads),
                    sn_b,
                )
                roped = rw.tile(
                    [128, nrope], BF16, tag=f"rp{nheads}", name="roped"
                )
                nc.vector.tensor_add(roped[:], t1[:], swp[:])
                return roped

            state = {"nxt": load_sc(0)}
            for c in range(NKC):
                eng = nc.sync if c % 2 == 0 else nc.scalar
                eng.dma_start(wt[c][:], w_d.ap()[:, c, :])
            m128 = consts.tile([128, 128], BF16, name="m128")
            nc.scalar.dma_start(m128[:], m128_d.ap())
            onec = consts.tile([128, 1], BF16, name="onec")
            nc.scalar.dma_start(onec[:], onec_d.ap())
            oner = consts.tile([1, 128], F32R, name="oner")
            nc.scalar.dma_start(oner[:], oner_d.ap())
            for hh in range(HL):
                nc.scalar.dma_start(wo[:, hh, :], wo_d.ap()[:, hh, :])

            def project_sc_gen(sc):
                """QKV projection + RoPE + transposes for sequence chunk sc.

                Yields after each contraction-chunk matmul pair so the caller
                can interleave these PE-only ops between attention tiles
                (which are paced by ACT's exp) to keep the PE fed.
                """
                qc = sc // 4
                xt, sn, cpm = state["nxt"]
                if sc + 1 < NSC:
                    state["nxt"] = load_sc(sc + 1)
                ph0 = psA.tile([128, NQ], F32, tag="ph", name="ph0")
                ph1 = psA.tile([128, 2 * NKV], F32, tag="ph", name="ph1")
                for c in range(NKC):
                    nc.tensor.matmul(
                        ph0[:], xt[:, c, :], wt[c][:, 0:NQ],
                        start=(c == 0), stop=(c == NKC - 1),
                    )
                    nc.tensor.matmul(
                        ph1[:], xt[:, c, :], wt[c][:, NQ:NW],
                        start=(c == 0), stop=(c == NKC - 1),
                    )
                    yield
                ropq = rope_half(ph0, HL, sn, cpm)
                nc.sync.dma_start_transpose(
                    qT[qc][:, :, (sc % 4) * 128:(sc % 4 + 1) * 128], ropq[:]
                )
                ropk = rope_half(ph1, KVL, sn, cpm)
                nc.sync.dma_start_transpose(
                    kT[qc][:, :, (sc % 4) * 128:(sc % 4 + 1) * 128], ropk[:]
                )
                nc.vector.tensor_copy(vn[sc][:], ph1[:, NKV:2 * NKV])

            def project_sc(sc):
                for _ in project_sc_gen(sc):
                    pass

            def attention(h, qc, filler=None):
                j = h // 2
                kend = 4 * qc + 4
                pv = psC.tile([128, 512], F32, tag="pv", name="pv")
                acc = accp.tile([128, 512], BF16, tag="acc", name="acc")
                if qc == NQC - 1:
                    nfeed = 2 if h == 0 else 1
                elif h < HL - 1:
                    nfeed = {0: 5, 1: 3}.get(qc, 2)
                else:
                    nfeed = 1
                for ki, kc in enumerate(range(kend)):
                    if filler is not None:
                        filler.feed(nfeed)
                    q0 = 128 * (kc - 4 * qc) if kc >= 4 * qc else 0
                    sp = psB.tile([128, 512], F32, tag="sp", name="sp")
                    nc.tensor.matmul(
                        sp[:, q0:512],
                        kT[kc // 4][:, j, (kc % 4) * 128:(kc % 4 + 1) * 128],
                        qT[qc][:, h, q0:512],
                        start=True, stop=True,
                    )
                    pe = pep.tile([128, 512], BF16, tag="pe", name="pe")
                    nc.scalar.activation(
                        pe[:, q0:512], sp[:, q0:512], EXP, scale=SCALE
                    )
                    if kc >= 4 * qc:
                        nc.vector.tensor_mul(
                            pe[:, q0:q0 + 128], pe[:, q0:q0 + 128], m128[:]
                        )
                    nc.tensor.matmul(
                        pv[:, q0:512],
                        vn[kc][:, j * 128:(j + 1) * 128],
                        pe[:, q0:512],
                        start=(ki == 0), stop=(ki == kend - 1),
                        skip_group_check=True,
                    )
                    if ki == 0:
                        nc.vector.tensor_copy(acc[:], pe[:])
                    else:
                        nc.vector.tensor_add(
                            acc[:, q0:512], acc[:, q0:512], pe[:, q0:512]
                        )
                if filler is not None:
                    filler.feed(2)
                rbb = psD.tile([128, 512], F32, tag="rbb", name="rbb")
                nc.tensor.matmul(
                    rbb[0:1, :], onec[:], acc[:], start=True, stop=True
                )
                rs_sb = rip.tile([1, 512], F32R, tag="ri", name="rs_sb")
                nc.scalar.copy(rs_sb[:], rbb[0:1, :])
                if filler is not None:
                    filler.feed(1)
                nc.tensor.matmul(rbb[:], oner[:], rs_sb[:], start=True, stop=True)
                rb = rip.tile([128, 512], F32, tag="rb", name="rb")
                nc.vector.reciprocal_approx_fast(out=rb[:], in_=rbb[:])
                aT = ares.tile([128, 512], BF16, tag=f"aT{h}", name=f"aT{h}")
                nc.vector.tensor_mul(aT[:], pv[:], rb[:])
                return aT

            def p3_gen(qc, aTc):
                """Output projection for q-panel qc; yields per (qp, dc) unit."""
                idx = 0
                for qp in range(4 * qc, 4 * qc + 4):
                    qoff = (qp % 4) * 128
                    for dc in range(4):
                        idx += 1
                        po = psB.tile([128, 512], F32, tag="sp", name="po")
                        for h in range(HL):
                            nc.tensor.matmul(
                                po[:],
                                aTc[h][:, qoff:qoff + 128],
                                wo[:, h, dc * 512:(dc + 1) * 512],
                                start=(h == 0), stop=(h == HL - 1),
                            )
                        osb = osp.tile([128, 512], BF16, tag="osb", name="osb")
                        on_act = (
                            (qp + dc) % 2 == 0 if qc == NQC - 1 else idx > 8
                        )
                        if on_act:
                            nc.scalar.copy(osb[:], po[:])
                        else:
                            nc.vector.tensor_copy(osb[:], po[:])
                        oeng = (
                            nc.sync if qc == NQC - 1 and (qp + dc) % 2 == 0
                            else nc.gpsimd
                        )
                        oeng.dma_start(
                            out_d.ap()[qp * 128:(qp + 1) * 128,
                                       dc * 512:(dc + 1) * 512],
                            osb[:],
                        )
                        yield

            class FillerQueue:
                def __init__(self):
                    self.gens = []

                def add(self, gen):
                    self.gens.append(gen)

                def feed(self, k=1):
                    for _ in range(k):
                        while self.gens:
                            try:
                                next(self.gens[0])
                                break
                            except StopIteration:
                                self.gens.pop(0)
                        else:
                            return

                def drain(self):
                    while self.gens:
                        for _ in self.gens.pop(0):
                            pass

            fq = FillerQueue()
            deferred = []
            for sc in range(4):
                project_sc(sc)
            for qc in range(NQC):
                if qc == NQC - 1:
                    for g in deferred:
                        fq.add(g)
                    deferred = []
                aTc = {}
                for h in range(HL):
                    if qc + 1 < NQC:
                        fq.add(project_sc_gen(4 * (qc + 1) + h))
                    aTc[h] = attention(h, qc, fq)
                fq.drain()
                if qc in (1, 2):
                    deferred.append(p3_gen(qc, aTc))
                else:
                    fq.add(p3_gen(qc, aTc))
            fq.drain()

    nc.compile()
    return nc


def _get_nc():
    global _BUILT
    if _BUILT is None:
        _BUILT = _build()
    return _BUILT


def _host_prep(x, freqs_cis, wq, wk, wv, wo):
    """Build the 8 per-core input maps (bf16)."""
    bf = ml_dtypes.bfloat16
    f = np.asarray(freqs_cis, dtype=np.float32)
    sn = np.repeat(f[:, :, 1], 2, axis=1)                    # (S, HD)
    cos = f[:, :, 0]
    cpm = np.empty((S, HD), dtype=np.float32)
    cpm[:, 0::2] = -cos
    cpm[:, 1::2] = cos
    sn_t = np.ascontiguousarray(sn.reshape(NSC, 128, HD)).astype(bf)
    cpm_t = np.ascontiguousarray(cpm.reshape(NSC, 128, HD)).astype(bf)

    kp = np.arange(128)[:, None]
    qf = np.arange(128)[None, :]
    m128 = (kp <= qf).astype(np.float32).astype(bf)           # (128,128)

    onec = np.ones((128, 1), dtype=np.float32).astype(bf)
    oner = np.ones((1, 128), dtype=np.float32)

    xts = []
    for b in range(DP):
        xb = np.asarray(x[b], dtype=np.float32)              # (S, D)
        x4 = xb.reshape(NSC, 128, NKC, 128).transpose(0, 3, 2, 1)
        xts.append(np.ascontiguousarray(x4).astype(bf))      # (NSC,128,NKC,128)

    wq = np.asarray(wq, dtype=np.float32)
    wk = np.asarray(wk, dtype=np.float32)
    wv = np.asarray(wv, dtype=np.float32)
    wo = np.asarray(wo, dtype=np.float32)

    in_maps = []
    for c in range(8):
        b, g = c // TP, c % TP
        w_all = np.concatenate(
            [
                wq[:, g * NQ:(g + 1) * NQ],
                wk[:, g * NKV:(g + 1) * NKV],
                wv[:, g * NKV:(g + 1) * NKV],
            ],
            axis=1,
        )  # (D, NW)
        w_t = np.ascontiguousarray(
            w_all.reshape(NKC, 128, NW).transpose(1, 0, 2)
        ).astype(bf)
        wo_g = wo[g * NQ:(g + 1) * NQ, :]                    # (NQ, D)
        wo_t = np.ascontiguousarray(
            wo_g.reshape(HL, 128, D).transpose(1, 0, 2)
        ).astype(bf)
        in_maps.append(
            {
                "xt": xts[b],
                "w": w_t,
                "wo": wo_t,
                "sn": sn_t,
                "cpm": cpm_t,
                "m128": m128,
                "onec": onec,
                "oner": oner,
            }
        )
    return in_maps


def kernel(x, freqs_cis, mask, wq, wk, wv, wo, _trace=False, _tmpdir=None):
    nc = _get_nc()
    in_maps = _host_prep(x, freqs_cis, wq, wk, wv, wo)
    res = run_bass_kernel_spmd(
        nc, in_maps, core_ids=list(range(8)), trace=_trace, tmpdir=_tmpdir
    )
    out = np.empty((B, S, D), dtype=np.float32)
    for b in range(DP):
        acc = res.results[b * TP + 0]["out"].astype(np.float32)
        for g in range(1, TP):
            acc = acc + res.results[b * TP + g]["out"].astype(np.float32)
        out[b] = acc
    kernel._last_results = res
    return out


# revision 29
# speedup vs baseline: 1.0141x; 1.0087x over previous
"""Trainium2 Bass kernel for GQA attention (B=2, S=2048, D=2048, H=16, KVH=8, HD=128).

Sharding: tensor-parallel over heads (4 groups of 4 q-heads / 2 kv-heads) x
data-parallel over batch (2) = 8 cores. Each core computes a partial output
(full rows for its batch, its head-group's contribution through wo); the host
sums the 4 partials per batch.

v3 dataflow (all matmul operands bf16, PSUM f32):
  - Software-pipelined: attention for q-block qc interleaves with the QKV
    projection of q-block qc+1 (one sequence chunk per head), then the output
    projection for qc's panel.
  - Each roped half transposes to head-dim-major layout with ONE DMA-xbar
    transpose (3D out AP maps logical row h*128+hd, matching the natural
    (h, hd) column order), dispatched on the scalar queue so the sync queue
    stays free for loads/stores.
  - RoPE on DVE straight out of PSUM (bf16 out); V natural via DVE copy.
  - Attention transposed: S^T tiles = K-blk stationary @ Q^T moving, exp on
    ACT (scale folded), causal handled by computing only the valid q-suffix
    of diagonal tiles + one [128,128] triangle mask multiply on DVE.
  - Softmax denominators accumulated on DVE (bf16); one ones-vector matmul
    folds partitions, ACT copy + PE broadcast + DVE reciprocal + multiply.
  - PV accumulated in PSUM with V stationary, ascending k so the first
    (full-width) tile opens the accumulation group.
  - bf16 output partials, summed in f32 on the host.
"""

import math
import sys
import types

import numpy as np
import ml_dtypes

import concourse.bass as bass
import concourse.mybir as mybir
import concourse.tile as tile
from concourse import bacc
from concourse.bass_utils import run_bass_kernel_spmd


def _install_ntff_hook():
    """Shim antenv.axon_hooks if the image lacks it, so trace=True (or a
    BASS_TRACE=1 environment) doesn't crash run_bass_kernel_spmd."""
    try:
        import antenv.axon_hooks  # noqa: F401

        return
    except ImportError:
        pass
    try:
        import antenv
        from trn_agent_boot.trn_boot import _ntff_profile_via_ctypes

        hook = _ntff_profile_via_ctypes("/opt/axon/libaxon_pjrt.so")
        mod = types.ModuleType("antenv.axon_hooks")
        state = {"hook": hook}
        mod.get_axon_ntff_profile_hook = lambda: state["hook"]
        mod.set_axon_ntff_profile_hook = lambda h: state.update(hook=h)
        sys.modules["antenv.axon_hooks"] = mod
        antenv.axon_hooks = mod
    except Exception:
        pass


_install_ntff_hook()

F32 = mybir.dt.float32
F32R = mybir.dt.float32r
BF16 = mybir.dt.bfloat16

B, S, D = 2, 2048, 2048
H, KVH, HD = 16, 8, 128
TP, DP = 4, 2
HL = H // TP        # 4 q heads per core
KVL = KVH // TP     # 2 kv heads per core
NQ = HL * HD        # 512 q cols per core
NKV = KVL * HD      # 256 k (and v) cols per core
NW = NQ + 2 * NKV   # 1024 fused qkv cols per core
NSC = S // 128      # 16 sequence chunks of 128
NKC = D // 128      # 16 contraction chunks of 128
NQC = S // 512      # 4 q chunks of 512
SCALE = 1.0 / math.sqrt(HD)

_BUILT = None


def _build():
    nc = bacc.Bacc("TRN2", target_bir_lowering=False, debug=False)

    xt_d = nc.dram_tensor("xt", (NSC, 128, NKC, 128), BF16, kind="ExternalInput")
    w_d = nc.dram_tensor("w", (128, NKC, NW), BF16, kind="ExternalInput")
    wo_d = nc.dram_tensor("wo", (128, HL, D), BF16, kind="ExternalInput")
    sn_d = nc.dram_tensor("sn", (NSC, 128, HD), BF16, kind="ExternalInput")
    cpm_d = nc.dram_tensor("cpm", (NSC, 128, HD), BF16, kind="ExternalInput")
    m128_d = nc.dram_tensor("m128", (128, 128), BF16, kind="ExternalInput")
    onec_d = nc.dram_tensor("onec", (128, 1), BF16, kind="ExternalInput")
    oner_d = nc.dram_tensor("oner", (1, 128), F32R, kind="ExternalInput")
    out_d = nc.dram_tensor("out", (S, D), BF16, kind="ExternalOutput")

    EXP = mybir.ActivationFunctionType.Exp

    with tile.TileContext(nc) as tc:
        with (
            nc.allow_low_precision(reason="bf16 dataflow is intentional"),
            tc.tile_pool(name="consts", bufs=1) as consts,
            tc.tile_pool(name="res", bufs=1) as res,
            tc.tile_pool(name="ares", bufs=2) as ares,
            tc.tile_pool(name="xtp", bufs=3) as xtp,
            tc.tile_pool(name="tabs", bufs=3) as tabs,
            tc.tile_pool(name="rw", bufs=2) as rw,
            tc.tile_pool(name="pep", bufs=6) as pep,
            tc.tile_pool(name="accp", bufs=2) as accp,
            tc.tile_pool(name="rip", bufs=2) as rip,
            tc.tile_pool(name="osp", bufs=3) as osp,
            tc.tile_pool(name="psA", bufs=2, space="PSUM") as psA,
            tc.tile_pool(name="psB", bufs=3, space="PSUM") as psB,
            tc.tile_pool(name="psC", bufs=2, space="PSUM") as psC,
            tc.tile_pool(name="psD", bufs=1, space="PSUM") as psD,
        ):

            wt = {
                c: consts.tile([128, NW], BF16, name=f"w_{c}")
                for c in range(NKC)
            }
            wo = consts.tile([128, HL, D], BF16, name="wo_t")

            # Residents: head-dim-major Q^T/K^T per q-block, natural V rows.
            qT = {}
            kT = {}
            for qc in range(NQC):
                qT[qc] = res.tile(
                    [128, HL, 512], BF16, tag=f"qT_{qc}", name=f"qT_{qc}"
                )
                kT[qc] = res.tile(
                    [128, KVL, 512], BF16, tag=f"kT_{qc}", name=f"kT_{qc}"
                )
            vn = {}
            for sc in range(NSC):
                vn[sc] = res.tile([128, NKV], BF16, tag=f"v_{sc}", name=f"v_{sc}")

            def load_sc(sc):
                xt = xtp.tile([128, NKC, 128], BF16, tag="xt", name="xt")
                eng = nc.gpsimd if sc > 0 else nc.sync
                eng.dma_start(xt[:, 0:8, :], xt_d.ap()[sc][:, 0:8, :])
                eng.dma_start(xt[:, 8:16, :], xt_d.ap()[sc][:, 8:16, :])
                sn = tabs.tile([128, HD], BF16, tag="sn", name="sn")
                nc.sync.dma_start(sn[:], sn_d.ap()[sc])
                cpm = tabs.tile([128, HD], BF16, tag="cpm", name="cpm")
                nc.sync.dma_start(cpm[:], cpm_d.ap()[sc])
                return xt, sn, cpm

            def rope_half(ph, nheads, sn, cpm):
                """RoPE the first nheads*HD cols (head-minor packed) -> bf16."""
                nrope = nheads * HD
                ph4 = ph[:, 0:nrope].rearrange(
                    "p (h i two) -> p h i two", h=nheads, two=2
                )
                cpm3 = cpm[:].rearrange("p (i two) -> p i two", two=2)
                cpm_e = cpm3[:, :, 0].unsqueeze(1).broadcast_to(
                    [128, nheads, HD // 2]
                )
                cpm_o = cpm3[:, :, 1].unsqueeze(1).broadcast_to(
                    [128, nheads, HD // 2]
                )
                sn_b = sn[:].unsqueeze(1).broadcast_to([128, nheads, HD])
                swp = rw.tile([128, nrope], BF16, tag=f"swp{nheads}", name="swp")
                swp4 = swp[:].rearrange(
                    "p (h i two) -> p h i two", h=nheads, two=2
                )
                nc.vector.tensor_mul(swp4[:, :, :, 0], ph4[:, :, :, 1], cpm_e)
                nc.vector.tensor_mul(swp4[:, :, :, 1], ph4[:, :, :, 0], cpm_o)
                t1 = rw.tile([128, nrope], BF16, tag=f"t1{nheads}", name="t1")
                nc.vector.tensor_mul(
                    t1[:].rearrange("p (h d) -> p h d", h=nheads),
                    ph[:, 0:nrope].rearrange("p (h d) -> p h d", h=nheads),
                    sn_b,
                )
                roped = rw.tile(
                    [128, nrope], BF16, tag=f"rp{nheads}", name="roped"
                )
                nc.vector.tensor_add(roped[:], t1[:], swp[:])
                return roped

            state = {"nxt": load_sc(0)}
            for c in range(NKC):
                eng = nc.sync if c % 2 == 0 else nc.scalar
                eng.dma_start(wt[c][:], w_d.ap()[:, c, :])
            m128 = consts.tile([128, 128], BF16, name="m128")
            nc.scalar.dma_start(m128[:], m128_d.ap())
            onec = consts.tile([128, 1], BF16, name="onec")
            nc.scalar.dma_start(onec[:], onec_d.ap())
            oner = consts.tile([1, 128], F32R, name="oner")
            nc.scalar.dma_start(oner[:], oner_d.ap())
            for hh in range(HL):
                nc.scalar.dma_start(wo[:, hh, :], wo_d.ap()[:, hh, :])

            def project_sc_gen(sc):
                """QKV projection + RoPE + transposes for sequence chunk sc.

                Yields after each contraction-chunk matmul pair so the caller
                can interleave these PE-only ops between attention tiles
                (which are paced by ACT's exp) to keep the PE fed.
                """
                qc = sc // 4
                xt, sn, cpm = state["nxt"]
                if sc + 1 < NSC:
                    state["nxt"] = load_sc(sc + 1)
                ph0 = psA.tile([128, NQ], F32, tag="ph", name="ph0")
                ph1 = psA.tile([128, 2 * NKV], F32, tag="ph", name="ph1")
                for c in range(NKC):
                    nc.tensor.matmul(
                        ph0[:], xt[:, c, :], wt[c][:, 0:NQ],
                        start=(c == 0), stop=(c == NKC - 1),
                    )
                    nc.tensor.matmul(
                        ph1[:], xt[:, c, :], wt[c][:, NQ:NW],
                        start=(c == 0), stop=(c == NKC - 1),
                    )
                    yield
                ropq = rope_half(ph0, HL, sn, cpm)
                nc.sync.dma_start_transpose(
                    qT[qc][:, :, (sc % 4) * 128:(sc % 4 + 1) * 128], ropq[:]
                )
                ropk = rope_half(ph1, KVL, sn, cpm)
                nc.sync.dma_start_transpose(
                    kT[qc][:, :, (sc % 4) * 128:(sc % 4 + 1) * 128], ropk[:]
                )
                nc.vector.tensor_copy(vn[sc][:], ph1[:, NKV:2 * NKV])

            def project_sc(sc):
                for _ in project_sc_gen(sc):
                    pass

            def attention(h, qc, filler=None):
                j = h // 2
                kend = 4 * qc + 4
                pv = psC.tile([128, 512], F32, tag="pv", name="pv")
                acc = accp.tile([128, 512], BF16, tag="acc", name="acc")
                if qc == NQC - 1:
                    nfeed = 2 if h == 0 else 1
                elif h < HL - 1:
                    nfeed = {0: 5, 1: 3}.get(qc, 2)
                else:
                    nfeed = 1
                for ki, kc in enumerate(range(kend)):
                    if filler is not None:
                        filler.feed(nfeed)
                    q0 = 128 * (kc - 4 * qc) if kc >= 4 * qc else 0
                    sp = psB.tile([128, 512], F32, tag="sp", name="sp")
                    nc.tensor.matmul(
                        sp[:, q0:512],
                        kT[kc // 4][:, j, (kc % 4) * 128:(kc % 4 + 1) * 128],
                        qT[qc][:, h, q0:512],
                        start=True, stop=True,
                    )
                    pe = pep.tile([128, 512], BF16, tag="pe", name="pe")
                    nc.scalar.activation(
                        pe[:, q0:512], sp[:, q0:512], EXP, scale=SCALE
                    )
                    if kc >= 4 * qc:
                        nc.vector.tensor_mul(
                            pe[:, q0:q0 + 128], pe[:, q0:q0 + 128], m128[:]
                        )
                    nc.tensor.matmul(
                        pv[:, q0:512],
                        vn[kc][:, j * 128:(j + 1) * 128],
                        pe[:, q0:512],
                        start=(ki == 0), stop=(ki == kend - 1),
                        skip_group_check=True,
                    )
                    if ki == 0:
                        nc.vector.tensor_copy(acc[:], pe[:])
                    else:
                        nc.vector.tensor_add(
                            acc[:, q0:512], acc[:, q0:512], pe[:, q0:512]
                        )
                if filler is not None:
                    filler.feed(2)
                rbb = psD.tile([128, 512], F32, tag="rbb", name="rbb")
                nc.tensor.matmul(
                    rbb[0:1, :], onec[:], acc[:], start=True, stop=True
                )
                rs_sb = rip.tile([1, 512], F32R, tag="ri", name="rs_sb")
                nc.scalar.copy(rs_sb[:], rbb[0:1, :])
                if filler is not None:
                    filler.feed(1)
                nc.tensor.matmul(rbb[:], oner[:], rs_sb[:], start=True, stop=True)
                rb = rip.tile([128, 512], F32, tag="rb", name="rb")
                nc.vector.reciprocal_approx_fast(out=rb[:], in_=rbb[:])
                aT = ares.tile([128, 512], BF16, tag=f"aT{h}", name=f"aT{h}")
                nc.vector.tensor_mul(aT[:], pv[:], rb[:])
                return aT

            def p3_gen(qc, aTc):
                """Output projection for q-panel qc; yields per (qp, dc) unit."""
                idx = 0
                for qp in range(4 * qc, 4 * qc + 4):
                    qoff = (qp % 4) * 128
                    for dc in range(4):
                        idx += 1
                        po = psB.tile([128, 512], F32, tag="sp", name="po")
                        for h in range(HL):
                            nc.tensor.matmul(
                                po[:],
                                aTc[h][:, qoff:qoff + 128],
                                wo[:, h, dc * 512:(dc + 1) * 512],
                                start=(h == 0), stop=(h == HL - 1),
                            )
                        osb = osp.tile([128, 512], BF16, tag="osb", name="osb")
                        on_act = (
                            (qp + dc) % 2 == 0 if qc == NQC - 1 else idx > 8
                        )
                        if on_act:
                            nc.scalar.copy(osb[:], po[:])
                        else:
                            nc.vector.tensor_copy(osb[:], po[:])
                        oeng = (
                            nc.sync if qc == NQC - 1 and (qp + dc) % 2 == 0
                            else nc.gpsimd
                        )
                        oeng.dma_start(
                            out_d.ap()[qp * 128:(qp + 1) * 128,
                                       dc * 512:(dc + 1) * 512],
                            osb[:],
                        )
                        yield

            class FillerQueue:
                def __init__(self):
                    self.gens = []

                def add(self, gen):
                    self.gens.append(gen)

                def feed(self, k=1):
                    for _ in range(k):
                        while self.gens:
                            try:
                                next(self.gens[0])
                                break
                            except StopIteration:
                                self.gens.pop(0)
                        else:
                            return

                def drain(self):
                    while self.gens:
                        for _ in self.gens.pop(0):
                            pass

            fq = FillerQueue()
            deferred = []
            for sc in range(4):
                project_sc(sc)
            for qc in range(NQC):
                if qc == NQC - 1:
                    for g in deferred:
                        fq.add(g)
                    deferred = []
                aTc = {}
                for h in range(HL):
                    if qc + 1 < NQC:
                        fq.add(project_sc_gen(4 * (qc + 1) + h))
                    aTc[h] = attention(h, qc, fq)
                fq.drain()
                if qc in (1, 2):
                    deferred.append(p3_gen(qc, aTc))
                else:
                    fq.add(p3_gen(qc, aTc))
            fq.drain()

    nc.compile()
    return nc


def _get_nc():
    global _BUILT
    if _BUILT is None:
        _BUILT = _build()
    return _BUILT


def _host_prep(x, freqs_cis, wq, wk, wv, wo):
    """Build the 8 per-core input maps (bf16)."""
    bf = ml_dtypes.bfloat16
    f = np.asarray(freqs_cis, dtype=np.float32)
    sn = np.repeat(f[:, :, 1], 2, axis=1)                    # (S, HD)
    cos = f[:, :, 0]
    cpm = np.empty((S, HD), dtype=np.float32)
    cpm[:, 0::2] = -cos
    cpm[:, 1::2] = cos
    sn_t = np.ascontiguousarray(sn.reshape(NSC, 128, HD)).astype(bf)
    cpm_t = np.ascontiguousarray(cpm.reshape(NSC, 128, HD)).astype(bf)

    kp = np.arange(128)[:, None]
    qf = np.arange(128)[None, :]
    m128 = (kp <= qf).astype(np.float32).astype(bf)           # (128,128)

    onec = np.ones((128, 1), dtype=np.float32).astype(bf)
    oner = np.ones((1, 128), dtype=np.float32)

    xts = []
    for b in range(DP):
        xb = np.asarray(x[b], dtype=np.float32)              # (S, D)
        x4 = xb.reshape(NSC, 128, NKC, 128).transpose(0, 3, 2, 1)
        xts.append(np.ascontiguousarray(x4).astype(bf))      # (NSC,128,NKC,128)

    wq = np.asarray(wq, dtype=np.float32)
    wk = np.asarray(wk, dtype=np.float32)
    wv = np.asarray(wv, dtype=np.float32)
    wo = np.asarray(wo, dtype=np.float32)

    in_maps = []
    for c in range(8):
        b, g = c // TP, c % TP
        w_all = np.concatenate(
            [
                wq[:, g * NQ:(g + 1) * NQ],
                wk[:, g * NKV:(g + 1) * NKV],
                wv[:, g * NKV:(g + 1) * NKV],
            ],
            axis=1,
        )  # (D, NW)
        w_t = np.ascontiguousarray(
            w_all.reshape(NKC, 128, NW).transpose(1, 0, 2)
        ).astype(bf)
        wo_g = wo[g * NQ:(g + 1) * NQ, :]                    # (NQ, D)
        wo_t = np.ascontiguousarray(
            wo_g.reshape(HL, 128, D).transpose(1, 0, 2)
        ).astype(bf)
        in_maps.append(
            {
                "xt": xts[b],
                "w": w_t,
                "wo": wo_t,
                "sn": sn_t,
                "cpm": cpm_t,
                "m128": m128,
                "onec": onec,
                "oner": oner,
            }
        )
    return in_maps


def kernel(x, freqs_cis, mask, wq, wk, wv, wo, _trace=False, _tmpdir=None):
    nc = _get_nc()
    in_maps = _host_prep(x, freqs_cis, wq, wk, wv, wo)
    res = run_bass_kernel_spmd(
        nc, in_maps, core_ids=list(range(8)), trace=_trace, tmpdir=_tmpdir
    )
    out = np.empty((B, S, D), dtype=np.float32)
    for b in range(DP):
        acc = res.results[b * TP + 0]["out"].astype(np.float32)
        for g in range(1, TP):
            acc = acc + res.results[b * TP + g]["out"].astype(np.float32)
        out[b] = acc
    kernel._last_results = res
    return out
